# revision 1
# baseline (speedup 1.0000x reference)
"""Trainium2 Bass kernel for nn_CustomCrossEntropyLoss_5368709120380.

loss = -mean_b log(y[b, t_b] + 1e-8) + sum_{b,c} w[t_b ^ c] * y[b,c] / (B*N)
where t_b = argmax_c target[b,c], w[k] = 6^popcount(k) (w[0] = 0).

Key algebraic trick: sum_c 6^popcount(c ^ t) * y[c] factorizes over bits,
so it is computed with a 10-stage halving butterfly per row:
    g' = (lo * r_k) + hi,  r_k = 6 if bit_k(t) else 1/6
followed by a correction factor P = 6^(10 - popcount(t)) (from using
r = a/b instead of exact (a,b) per stage), and subtracting the c == t
term (weight 6^0 = 1, but w[0] = 0).

Sharding: pure data parallel over the batch across 8 NeuronCores;
each core returns partial sums (pt_sum, ce_sum); host combines.

Self-contained: hardcodes B=65536, N=1024, 8 cores.
"""
import math

import numpy as np

import concourse.bacc as bacc
import concourse.bass as bass
import concourse.mybir as mybir
import concourse.tile as tile
from concourse.bass_utils import run_bass_kernel_spmd

F32 = mybir.dt.float32
U16 = mybir.dt.uint16
U32 = mybir.dt.uint32
AX = mybir.AxisListType
OP = mybir.AluOpType
ACT = mybir.ActivationFunctionType

B_FULL = 65536
N = 1024
DIM = 10
N_CORES = 8
B_SHARD = B_FULL // N_CORES          # 8192
N_TILES = B_SHARD // 128             # 64
LN6 = math.log(6.0)

_cache = {}


def _build_program():
    nc = bacc.Bacc("TRN2", target_bir_lowering=False, debug=False)
    y_d = nc.dram_tensor("y_true", [B_SHARD, N], F32, kind="ExternalInput")
    t_d = nc.dram_tensor("target", [B_SHARD, N], F32, kind="ExternalInput")
    cu_d = nc.dram_tensor("c_u32", [128, DIM], U32, kind="ExternalInput")
    cf_d = nc.dram_tensor("c_f32", [128, 27], F32, kind="ExternalInput")
    ci_d = nc.dram_tensor("c_iota", [128, N], F32, kind="ExternalInput")
    out_d = nc.dram_tensor("out", [1, 2], F32, kind="ExternalOutput")

    with tile.TileContext(nc) as tc:
        with (
            tc.tile_pool(name="const", bufs=1) as cpool,
            tc.tile_pool(name="io", bufs=4) as iopool,
            tc.tile_pool(name="small", bufs=6) as spool,
            tc.tile_pool(name="btf", bufs=3) as bpool,
            tc.tile_pool(name="strip", bufs=1) as stpool,
            tc.tile_pool(name="ps", bufs=1, space=bass.MemorySpace.PSUM) as pspool,
        ):
            pow2 = cpool.tile([128, DIM], U32)
            nc.sync.dma_start(pow2[:], cu_d[:])
            cf = cpool.tile([128, 27], F32)
            nc.sync.dma_start(cf[:], cf_d[:])
            iota = cpool.tile([128, N], F32)
            nc.sync.dma_start(iota[:], ci_d[:])
            diag = cf[:, 0:16]       # diag[p, i] = (i == p % 16)
            ones8 = cf[:, 16:24]     # 1.0
            ones1 = cf[:, 24:25]     # 1.0
            bias_exp = cf[:, 25:26]  # 10*ln6
            bias_ln = cf[:, 26:27]   # 1e-8

            pt_strip = stpool.tile([128, N_TILES], F32)
            ce_strip = stpool.tile([128, N_TILES], F32)
            ysel_strip = stpool.tile([128, N_TILES], F32)
            pc_strip = stpool.tile([128, N_TILES], F32)
            g10_strip = stpool.tile([128, N_TILES], F32)

            for i in range(N_TILES):
                ty = iopool.tile([128, N], F32, tag="y")
                nc.sync.dma_start(ty[:], y_d[i * 128:(i + 1) * 128, :])
                tt = iopool.tile([128, N], F32, tag="t")
                nc.sync.dma_start(tt[:], t_d[i * 128:(i + 1) * 128, :])

                # t_p = argmax_c target[p, c]  (first index on ties)
                rmax = spool.tile([128, 1], F32, tag="rmax")
                nc.vector.reduce_max(rmax[:], tt[:], axis=AX.X)
                rmax8 = spool.tile([128, 8], F32, tag="rmax8")
                nc.vector.tensor_scalar(rmax8[:], ones8, rmax[:, 0:1], None, OP.mult)
                idx = spool.tile([128, 8], U16, tag="idx")
                nc.vector.max_index(idx[:], rmax8[:], tt[:])

                # bits[p,k] = bit (9-k) of t_p, as f32 0/1
                idx32 = spool.tile([128, 1], U32, tag="idx32")
                nc.vector.tensor_copy(idx32[:], idx[:, 0:1])
                bits_u = spool.tile([128, DIM], U32, tag="bits_u")
                nc.vector.tensor_tensor(
                    bits_u[:], pow2[:], idx32[:, 0:1].to_broadcast((128, DIM)),
                    OP.bitwise_and,
                )
                bits = spool.tile([128, DIM], F32, tag="bits")
                nc.gpsimd.tensor_scalar(bits[:], bits_u[:], 1, None, OP.is_ge)
                # r[p,k] = 6 if bit else 1/6
                rr = spool.tile([128, DIM], F32, tag="rr")
                nc.gpsimd.tensor_scalar(
                    rr[:], bits[:], 6.0 - 1.0 / 6.0, 1.0 / 6.0, OP.mult, OP.add
                )
                nc.vector.reduce_sum(pc_strip[:, i:i + 1], bits[:], axis=AX.X)

                # gather y[p, t_p] = sum_c (iota == t) * y
                tf = spool.tile([128, 1], F32, tag="tf")
                nc.vector.tensor_copy(tf[:], idx[:, 0:1])
                oh = bpool.tile([128, N], F32, tag="oh")
                nc.gpsimd.tensor_scalar(oh[:], iota[:], tf[:, 0:1], None, OP.is_equal)
                scr = bpool.tile([128, N], F32, tag="scr")
                nc.vector.scalar_tensor_tensor(
                    scr[:], oh[:], 1.0, ty[:], OP.mult, OP.mult,
                    accum_out=ysel_strip[:, i:i + 1],
                )

                # butterfly
                # stage 0 split: ACT does lo*r0, Pool adds hi
                u0 = bpool.tile([128, 512], F32, tag="u0")
                nc.scalar.activation(
                    u0[:], ty[:, 0:512], ACT.Copy, bias=0.0, scale=rr[:, 0:1]
                )
                g = bpool.tile([128, 512], F32, tag="g0")
                nc.gpsimd.tensor_tensor(g[:], u0[:], ty[:, 512:1024], OP.add)
                prev = g
                L = 256
                k = 1
                while L >= 1:
                    if L == 1:
                        nxt = g10_strip[:, i:i + 1]
                    else:
                        nxt_t = bpool.tile([128, L], F32, tag=f"g{k}")
                        nxt = nxt_t[:]
                    nc.vector.scalar_tensor_tensor(
                        nxt, prev[:, 0:L], rr[:, k:k + 1], prev[:, L:2 * L],
                        OP.mult, OP.add,
                    )
                    prev = nxt
                    L //= 2
                    k += 1



            # batched epilogue: P = exp(10ln6 - ln6*pc), ce = ln(ysel+1e-8),
            # pt = g10*P - ysel  (single ACT table per function, 2 loads total)
            p_strip = stpool.tile([128, N_TILES], F32)
            nc.scalar.activation(p_strip[:], pc_strip[:], ACT.Exp, bias=bias_exp, scale=-LN6)
            nc.scalar.activation(ce_strip[:], ysel_strip[:], ACT.Ln, bias=bias_ln, scale=1.0)
            nc.vector.tensor_tensor(pt_strip[:], g10_strip[:], p_strip[:], OP.mult)
            nc.vector.tensor_tensor(pt_strip[:], pt_strip[:], ysel_strip[:], OP.subtract)

            ptsum = spool.tile([128, 1], F32, tag="ptsum")
            nc.vector.reduce_sum(ptsum[:], pt_strip[:], axis=AX.X)
            cesum = spool.tile([128, 1], F32, tag="cesum")
            nc.vector.reduce_sum(cesum[:], ce_strip[:], axis=AX.X)
            packed = spool.tile([128, 2], F32, tag="packed")
            nc.vector.tensor_copy(packed[:, 0:1], ptsum[:])
            nc.vector.tensor_copy(packed[:, 1:2], cesum[:])

            acc = pspool.tile([1, 2], F32)
            nc.tensor.matmul(acc[:], ones1, packed[:], start=True, stop=True)
            sb_out = spool.tile([1, 2], F32, tag="sbout")
            nc.vector.tensor_copy(sb_out[:], acc[:])
            nc.sync.dma_start(out_d[:], sb_out[:])

    nc.compile()
    return nc


def _consts():
    cu = np.zeros((128, DIM), dtype=np.uint32)
    cu[:] = (2 ** np.arange(DIM - 1, -1, -1, dtype=np.uint32))[None, :]
    cf = np.zeros((128, 27), dtype=np.float32)
    for p in range(128):
        cf[p, p % 16] = 1.0
    cf[:, 16:25] = 1.0
    cf[:, 25] = DIM * LN6
    cf[:, 26] = 1e-8
    ci = np.broadcast_to(np.arange(N, dtype=np.float32), (128, N)).copy()
    return cu, cf, ci


def kernel(y_true: np.ndarray, target: np.ndarray) -> np.ndarray:
    assert y_true.shape == (B_FULL, N) and target.shape == (B_FULL, N)
    if "nc" not in _cache:
        _cache["nc"] = _build_program()
    nc = _cache["nc"]

    cu, cf, ci = _consts()
    in_maps = []
    for c in range(N_CORES):
        sl = slice(c * B_SHARD, (c + 1) * B_SHARD)
        in_maps.append({
            "y_true": np.ascontiguousarray(y_true[sl]),
            "target": np.ascontiguousarray(target[sl]),
            "c_u32": cu,
            "c_f32": cf,
            "c_iota": ci,
        })

    res = run_bass_kernel_spmd(nc, in_maps, core_ids=list(range(N_CORES)))
    _cache["last_results"] = res

    pt_sum = 0.0
    ce_sum = 0.0
    for c in range(N_CORES):
        o = res.results[c]["out"]
        pt_sum += float(o[0, 0])
        ce_sum += float(o[0, 1])
    loss = -ce_sum / B_FULL + pt_sum / (B_FULL * N)
    return np.float32(loss)



# revision 7
# speedup vs baseline: 2.5401x; 2.5401x over previous
"""Trainium2 Bass kernel for nn_CustomCrossEntropyLoss_5368709120380.

loss = -mean_b log(y[b, t_b] + 1e-8) + sum_{b,c} w[t_b ^ c] * y[b,c] / (B*N)
where t_b = argmax_c target[b,c], w[k] = 6^popcount(k) (w[0] = 0).

Key restructure vs. the butterfly baseline: the XOR-popcount weight is
built as a TensorEngine matmul in log space.

    ln w[t_b ^ c] = ln6 * pc(t_b ^ c)
                  = sum_k bit_k(c) * ln6*(1 - 2*bit_k(t_b)) + ln6*pc(t_b)

so with a per-row vector v_b = [ln6*(1-2*b_0..9), ln6*pc(t_b)] (11 wide)
and a fixed matrix C[k, c] = bit_k(c) (row 10 = 1.0):

    E[b, c] = (v^T C)[b, c] = ln6 * pc(t_b ^ c)

We accumulate ln(y + 1e-8) into the same PSUM tile via an identity
matmul, so  exp(E + ln y) = w * y, and the row sum comes for free from
the Scalar engine's accum_out on the single Exp activation.  The c==t_b
term (weight 1) is removed by subtracting ysel = exp(ln y[t_b]), where
ln y[t_b] is fetched per-row with a GpSimd indirect_copy (diagonal of a
16-wide group gather).

Per 128-row tile: Vector does only argmax (max + max_index) and tiny
bit ops; Tensor does transpose + 4 matmuls; Scalar does Ln + Exp(+accum);
GpSimd does the 2 gather ops.  All engines sit below the DMA roofline.

Sharding: pure data parallel over batch across 8 NeuronCores; each core
returns partial sums (pt_sum, ce_sum); host combines.

Self-contained: hardcodes B=65536, N=1024, 8 cores.
"""
import math

import numpy as np

import concourse.bacc as bacc
import concourse.bass as bass
import concourse.mybir as mybir
import concourse.tile as tile
from concourse.bass_utils import run_bass_kernel_spmd

F32 = mybir.dt.float32
U16 = mybir.dt.uint16
AX = mybir.AxisListType
OP = mybir.AluOpType
ACT = mybir.ActivationFunctionType

B_FULL = 65536
N = 1024
DIM = 10
N_CORES = 8
B_SHARD = B_FULL // N_CORES          # 8192
N_TILES = B_SHARD // 128             # 64
LN6 = math.log(6.0)

_cache = {}


def _build_program():
    nc = bacc.Bacc("TRN2", target_bir_lowering=False, debug=False)
    y_d = nc.dram_tensor("y_true", [B_SHARD, N], F32, kind="ExternalInput")
    t_d = nc.dram_tensor("target", [B_SHARD, N], F32, kind="ExternalInput")
    pw_d = nc.dram_tensor("c_pow2", [128, DIM], U16, kind="ExternalInput")
    cb_d = nc.dram_tensor("c_bits", [DIM + 2, N], F32, kind="ExternalInput")
    id_d = nc.dram_tensor("c_ident", [128, 128], F32, kind="ExternalInput")
    cm_d = nc.dram_tensor("c_misc", [128, 48], F32, kind="ExternalInput")
    out_d = nc.dram_tensor("out", [1, 2], F32, kind="ExternalOutput")

    with tile.TileContext(nc) as tc:
        with (
            tc.tile_pool(name="const", bufs=1) as cpool,
            tc.tile_pool(name="io", bufs=4) as iopool,
            tc.tile_pool(name="lny", bufs=3) as lpool,
            tc.tile_pool(name="small", bufs=4) as spool,
            tc.tile_pool(name="scr", bufs=1) as scrpool,
            tc.tile_pool(name="strip", bufs=1) as stpool,
            tc.tile_pool(name="psE", bufs=2, space=bass.MemorySpace.PSUM) as pse,
            tc.tile_pool(name="psT", bufs=2, space=bass.MemorySpace.PSUM) as pst,
            tc.tile_pool(name="psO", bufs=1, space=bass.MemorySpace.PSUM) as pso,
        ):
            pw = cpool.tile([128, DIM], U16)
            nc.sync.dma_start(pw[:], pw_d[:])
            cb = cpool.tile([DIM + 2, N], F32)
            nc.sync.dma_start(cb[:], cb_d[:])
            ident = cpool.tile([128, 128], F32)
            nc.sync.dma_start(ident[:], id_d[:])
            misc = cpool.tile([128, 48], F32)
            nc.sync.dma_start(misc[:], cm_d[:])
            pow2 = pw[:, 0:DIM]          # 1 << k
            diag16 = misc[:, 0:16]       # diag16[p, i] = (i == p % 16)
            zeros10 = misc[:, 16:26]     # 0.0
            zero1 = misc[:, 26:27]       # 0.0 (Exp bias)
            eps1 = misc[:, 27:28]        # 1e-8 (Ln bias)
            ones1 = misc[:, 28:29]       # 1.0 (final matmul lhsT)

            pt_strip = stpool.tile([128, N_TILES], F32)
            ly_strip = stpool.tile([128, N_TILES], F32)

            eprev = None  # (E_psum, col) pending Exp, software-pipelined

            for i in range(N_TILES):
                ty = iopool.tile([128, N], F32, tag="y")
                nc.sync.dma_start(ty[:], y_d[i * 128:(i + 1) * 128, :])
                tt = iopool.tile([128, N], F32, tag="t")
                nc.sync.dma_start(tt[:], t_d[i * 128:(i + 1) * 128, :])

                # t_p = argmax_c target[p, c] (first index on ties)
                vmax8 = spool.tile([128, 8], F32, tag="vmax8")
                nc.vector.max(vmax8[:], tt[:])
                idx = spool.tile([128, 8], U16, tag="idx")
                nc.vector.max_index(idx[:], vmax8[:], tt[:])

                # v[:, k<10] = -2*ln6*bit_k(t), v[:, 10] = ln6*pc(t), v[:, 11] = 1
                bits_u = spool.tile([128, DIM], U16, tag="bits_u")
                nc.vector.tensor_tensor(
                    bits_u[:], pow2, idx[:, 0:1].to_broadcast((128, DIM)),
                    OP.bitwise_and,
                )
                vtile = spool.tile([128, DIM + 2], F32, tag="vtile")
                nc.vector.tensor_scalar(
                    vtile[:, 0:DIM], bits_u[:], 1, -2.0 * LN6, OP.is_ge, OP.mult
                )
                scr10 = spool.tile([128, DIM], F32, tag="scr10")
                nc.vector.scalar_tensor_tensor(
                    scr10[:], vtile[:, 0:DIM], -0.5, zeros10,
                    OP.mult, OP.add,
                    accum_out=vtile[:, DIM:DIM + 1],
                )
                nc.vector.tensor_copy(vtile[:, DIM + 1:DIM + 2], ones1)

                # vT = vtile^T via TensorE, staged through PSUM
                vT_ps = pst.tile([DIM + 2, 128], F32)
                nc.tensor.transpose(vT_ps[:], vtile[:], ident[:])
                vT = spool.tile([DIM + 2, 128], F32, tag="vT")
                nc.vector.tensor_copy(vT[:], vT_ps[:])

                # lny = ln(y + 1e-8)
                lny = lpool.tile([128, N], F32, tag="lny")
                nc.scalar.activation(lny[:], ty[:], ACT.Ln, bias=eps1, scale=1.0)

                # E = lny + ln6 * pc(t ^ c), via PSUM-accumulated matmuls
                e_ps = pse.tile([128, N], F32)
                nc.tensor.matmul(
                    e_ps[:, 0:512], ident[:], lny[:, 0:512],
                    start=True, stop=False,
                )
                nc.tensor.matmul(
                    e_ps[:, 0:512], vT[:], cb[:, 0:512],
                    start=False, stop=True,
                )
                nc.tensor.matmul(
                    e_ps[:, 512:N], ident[:], lny[:, 512:N],
                    start=True, stop=False,
                )
                nc.tensor.matmul(
                    e_ps[:, 512:N], vT[:], cb[:, 512:N],
                    start=False, stop=True,
                )

                # lnysel[p] = lny[p, t_p]: group-gather 16 then diagonal
                g16 = spool.tile([128, 16], F32, tag="g16")
                nc.gpsimd.indirect_copy(g16[:], lny[:], idx[:, 0:1], True)
                scr16 = scrpool.tile([128, 16], F32, tag="scr16")
                nc.vector.scalar_tensor_tensor(
                    scr16[:], g16[:], 1.0, diag16, OP.mult, OP.mult,
                    accum_out=ly_strip[:, i:i + 1],
                )

                # Exp of the previous tile's E (software-pipelined so the
                # Scalar engine's serial order Ln(i+1) -> Exp(i) never
                # stalls on the current tile's matmuls)
                if eprev is not None:
                    ep, j = eprev
                    scrE = scrpool.tile([128, N], F32, tag="scrE")
                    nc.scalar.activation(
                        scrE[:], ep[:], ACT.Exp, bias=zero1, scale=1.0,
                        accum_out=pt_strip[:, j:j + 1],
                    )
                eprev = (e_ps, i)

            ep, j = eprev
            scrE = scrpool.tile([128, N], F32, tag="scrE")
            nc.scalar.activation(
                scrE[:], ep[:], ACT.Exp, bias=zero1, scale=1.0,
                accum_out=pt_strip[:, j:j + 1],
            )

            # epilogue: pt_row -= ysel;  ce = sum lnysel
            ysel_strip = stpool.tile([128, N_TILES], F32)
            nc.scalar.activation(
                ysel_strip[:], ly_strip[:], ACT.Exp, bias=zero1, scale=1.0
            )
            nc.vector.tensor_tensor(
                pt_strip[:], pt_strip[:], ysel_strip[:], OP.subtract
            )

            ptsum = spool.tile([128, 1], F32, tag="ptsum")
            nc.vector.reduce_sum(ptsum[:], pt_strip[:], axis=AX.X)
            cesum = spool.tile([128, 1], F32, tag="cesum")
            nc.vector.reduce_sum(cesum[:], ly_strip[:], axis=AX.X)
            packed = spool.tile([128, 2], F32, tag="packed")
            nc.vector.tensor_copy(packed[:, 0:1], ptsum[:])
            nc.vector.tensor_copy(packed[:, 1:2], cesum[:])

            acc = pso.tile([1, 2], F32)
            nc.tensor.matmul(acc[:], ones1, packed[:], start=True, stop=True)
            sb_out = spool.tile([1, 2], F32, tag="sbout")
            nc.vector.tensor_copy(sb_out[:], acc[:])
            nc.sync.dma_start(out_d[:], sb_out[:])

    nc.compile()
    return nc


def _consts():
    pw = np.zeros((128, DIM), dtype=np.uint16)
    pw[:] = (1 << np.arange(DIM, dtype=np.uint16))[None, :]
    cbits = np.zeros((DIM + 2, N), dtype=np.float32)
    c = np.arange(N, dtype=np.uint32)
    pc = np.zeros(N, dtype=np.float32)
    for k in range(DIM):
        cbits[k, :] = (c >> k) & 1
        pc += cbits[k, :]
    cbits[DIM, :] = 1.0
    cbits[DIM + 1, :] = LN6 * pc
    ident = np.eye(128, dtype=np.float32)
    misc = np.zeros((128, 48), dtype=np.float32)
    for p in range(128):
        misc[p, p % 16] = 1.0
    misc[:, 16:26] = 0.0
    misc[:, 26] = 0.0
    misc[:, 27] = 1e-8
    misc[:, 28] = 1.0
    return pw, cbits, ident, misc


def kernel(y_true: np.ndarray, target: np.ndarray) -> np.ndarray:
    assert y_true.shape == (B_FULL, N) and target.shape == (B_FULL, N)
    if "nc" not in _cache:
        _cache["nc"] = _build_program()
    nc = _cache["nc"]

    pw, cbits, ident, misc = _consts()
    in_maps = []
    for c in range(N_CORES):
        sl = slice(c * B_SHARD, (c + 1) * B_SHARD)
        in_maps.append({
            "y_true": np.ascontiguousarray(y_true[sl]),
            "target": np.ascontiguousarray(target[sl]),
            "c_pow2": pw,
            "c_bits": cbits,
            "c_ident": ident,
            "c_misc": misc,
        })

    res = run_bass_kernel_spmd(nc, in_maps, core_ids=list(range(N_CORES)))
    _cache["last_results"] = res

    pt_sum = 0.0
    ce_sum = 0.0
    for c in range(N_CORES):
        o = res.results[c]["out"]
        pt_sum += float(o[0, 0])
        ce_sum += float(o[0, 1])
    loss = -ce_sum / B_FULL + pt_sum / (B_FULL * N)
    return np.float32(loss)


# revision 13
# speedup vs baseline: 4.2908x; 1.6893x over previous
"""Trainium2 Bass kernel for nn_CustomCrossEntropyLoss_5368709120380.

loss = -mean_b log(y[b, t_b] + 1e-8) + sum_{b,c} w[t_b ^ c] * y[b,c] / (B*N)
where t_b = argmax_c target[b,c], w[k] = 6^popcount(k) (w[0] = 0).

Key restructure vs. the butterfly baseline: the XOR-popcount exponent is
built as a TensorEngine matmul over exact small integers:

    pc(t_b ^ c) = sum_k bit_k(c) * (-2*bit_k(t_b)) + pc(t_b) + pc(c)

so with a per-row vector v_b = [-2*b_0.., pc(t_b), 1] (12 wide) and a
fixed bf16 matrix C (rows: bit_k(c), 1.0, pc(c)) -- all values exact in
bf16 -- one 12-deep matmul per 128-row tile gives E[b,c] = pc(t_b ^ c)
in PSUM.  The Scalar engine computes K = exp(E * ln6) = 6^pc (ln6 via
the activation's immediate scale, so no f32 matmul is needed), GpSimd
multiplies K*y, and a Copy activation with accum_out row-sums it.
The c==t_b term (weight 6^0=1, but w[0]=0) is removed by subtracting
ysel = y[t_b], fetched per-row with a GpSimd indirect_copy (diagonal of
a 16-wide group gather).

Per 128-row tile: Vector does only argmax (max + max_index) and tiny
bit ops; Tensor does transpose + 2 bf16 matmuls; Scalar does Exp and
Copy+accum (one act table, no reloads); GpSimd does the elementwise
multiply and the 2 gather ops.  All engines sit below the DMA roofline.

Sharding: pure data parallel over batch across 8 NeuronCores; each core
returns partial sums (pt_sum, ce_sum); host combines.

Self-contained: hardcodes B=65536, N=1024, 8 cores.
"""
import math

import numpy as np

import concourse.bacc as bacc
import concourse.bass as bass
import concourse.mybir as mybir
import concourse.tile as tile
from concourse.bass_utils import run_bass_kernel_spmd

F32 = mybir.dt.float32
BF16 = mybir.dt.bfloat16
U16 = mybir.dt.uint16
AX = mybir.AxisListType
OP = mybir.AluOpType
ACT = mybir.ActivationFunctionType

B_FULL = 65536
N = 1024
DIM = 10
N_CORES = 8
B_SHARD = B_FULL // N_CORES          # 8192
N_TILES = B_SHARD // 128             # 64
LN6 = math.log(6.0)

_cache = {}


def _build_program():
    nc = bacc.Bacc("TRN2", target_bir_lowering=False, debug=False)
    y_d = nc.dram_tensor("y_true", [B_SHARD, N], F32, kind="ExternalInput")
    t_d = nc.dram_tensor("target", [B_SHARD, N], F32, kind="ExternalInput")
    pw_d = nc.dram_tensor("c_pow2", [128, DIM], U16, kind="ExternalInput")
    cb_d = nc.dram_tensor("c_bits", [DIM + 2, N], BF16, kind="ExternalInput")
    id_d = nc.dram_tensor("c_ident", [128, 128], F32, kind="ExternalInput")
    cm_d = nc.dram_tensor("c_misc", [128, 48], F32, kind="ExternalInput")
    out_d = nc.dram_tensor("out", [1, 2], F32, kind="ExternalOutput")

    with tile.TileContext(nc) as tc:
        with (
            tc.tile_pool(name="const", bufs=1) as cpool,
            tc.tile_pool(name="io", bufs=4) as iopool,
            tc.tile_pool(name="lny", bufs=3) as lpool,
            tc.tile_pool(name="small", bufs=4) as spool,
            tc.tile_pool(name="scr", bufs=1) as scrpool,
            tc.tile_pool(name="strip", bufs=1) as stpool,
            tc.tile_pool(name="psE", bufs=2, space=bass.MemorySpace.PSUM) as pse,
            tc.tile_pool(name="psT", bufs=2, space=bass.MemorySpace.PSUM) as pst,
            tc.tile_pool(name="psO", bufs=1, space=bass.MemorySpace.PSUM) as pso,
        ):
            pw = cpool.tile([128, DIM], U16)
            nc.sync.dma_start(pw[:], pw_d[:])
            cb = cpool.tile([DIM + 2, N], BF16)
            nc.sync.dma_start(cb[:], cb_d[:])
            ident = cpool.tile([128, 128], F32)
            nc.sync.dma_start(ident[:], id_d[:])
            misc = cpool.tile([128, 48], F32)
            nc.sync.dma_start(misc[:], cm_d[:])
            pow2 = pw[:, 0:DIM]          # 1 << k
            diag16 = misc[:, 0:16]       # diag16[p, i] = (i == p % 16)
            zeros10 = misc[:, 16:26]     # 0.0
            zero1 = misc[:, 26:27]       # 0.0 (Exp bias)
            eps1 = misc[:, 27:28]        # 1e-8 (Ln bias)
            ones1 = misc[:, 28:29]       # 1.0 (final matmul lhsT)

            pt_strip = stpool.tile([128, N_TILES], F32)
            ys_strip = stpool.tile([128, N_TILES], F32)

            exp_q = []  # (E_psum, col) pending Exp, software-pipelined
            mul_q = []  # (K, ty, col) pending GpSimd multiply
            acc_q = []  # (Z, col) pending Copy+accum row-sum

            def drain(min_exp, min_mul, min_acc):
                while len(exp_q) > min_exp:
                    ep, j = exp_q.pop(0)
                    kt = lpool.tile([128, N], F32, tag="k")
                    nc.scalar.activation(
                        kt[:], ep[:], ACT.Exp, bias=zero1, scale=LN6
                    )
                    mul_q.append((kt, ty_ring[j % 4], j))
                while len(mul_q) > min_mul:
                    kt, tyj, j = mul_q.pop(0)
                    zt = lpool.tile([128, N], F32, tag="z")
                    nc.gpsimd.tensor_tensor(zt[:], kt[:], tyj[:], OP.mult)
                    acc_q.append((zt, j))
                while len(acc_q) > min_acc:
                    zt, j = acc_q.pop(0)
                    scrE = scrpool.tile([128, N], F32, tag="scrE")
                    nc.scalar.activation(
                        scrE[:], zt[:], ACT.Copy, bias=0.0, scale=1.0,
                        accum_out=pt_strip[:, j:j + 1],
                    )

            ty_ring = {}

            for i in range(N_TILES):
                ty = iopool.tile([128, N], F32, tag="y")
                nc.sync.dma_start(ty[:], y_d[i * 128:(i + 1) * 128, :])
                ty_ring[i % 4] = ty
                tt = iopool.tile([128, N], F32, tag="t")
                nc.sync.dma_start(tt[:], t_d[i * 128:(i + 1) * 128, :])

                # t_p = argmax_c target[p, c] (first index on ties)
                vmax8 = spool.tile([128, 8], F32, tag="vmax8")
                nc.vector.max(vmax8[:], tt[:])
                idx = spool.tile([128, 8], U16, tag="idx")
                nc.vector.max_index(idx[:], vmax8[:], tt[:])

                # v[:, k<10] = -2*bit_k(t), v[:, 10] = pc(t), v[:, 11] = 1
                bits_u = spool.tile([128, DIM], U16, tag="bits_u")
                nc.vector.tensor_tensor(
                    bits_u[:], pow2, idx[:, 0:1].to_broadcast((128, DIM)),
                    OP.bitwise_and,
                )
                vtile = spool.tile([128, DIM + 2], F32, tag="vtile")
                nc.vector.tensor_scalar(
                    vtile[:, 0:DIM], bits_u[:], 1, -2.0, OP.is_ge, OP.mult
                )
                scr10 = spool.tile([128, DIM], F32, tag="scr10")
                nc.vector.scalar_tensor_tensor(
                    scr10[:], vtile[:, 0:DIM], -0.5, zeros10,
                    OP.mult, OP.add,
                    accum_out=vtile[:, DIM:DIM + 1],
                )
                nc.vector.tensor_copy(vtile[:, DIM + 1:DIM + 2], ones1)

                # vT = vtile^T via TensorE, staged through PSUM, cast to bf16
                vT_ps = pst.tile([DIM + 2, 128], F32)
                nc.tensor.transpose(vT_ps[:], vtile[:], ident[:])
                vT = spool.tile([DIM + 2, 128], BF16, tag="vT")
                nc.vector.tensor_copy(vT[:], vT_ps[:])

                # E[b, c] = pc(t_b ^ c), two bf16 matmuls
                e_ps = pse.tile([128, N], F32)
                nc.tensor.matmul(
                    e_ps[:, 0:512], vT[:], cb[:, 0:512], start=True, stop=True
                )
                nc.tensor.matmul(
                    e_ps[:, 512:N], vT[:], cb[:, 512:N], start=True, stop=True
                )
                exp_q.append((e_ps, i))

                # ysel[p] = y[p, t_p]: group-gather 16 then diagonal
                g16 = spool.tile([128, 16], F32, tag="g16")
                nc.gpsimd.indirect_copy(g16[:], ty[:], idx[:, 0:1], True)
                scr16 = scrpool.tile([128, 16], F32, tag="scr16")
                nc.vector.scalar_tensor_tensor(
                    scr16[:], g16[:], 1.0, diag16, OP.mult, OP.mult,
                    accum_out=ys_strip[:, i:i + 1],
                )

                # K = 6^E on Scalar, Z = K*y on GpSimd, row-sum via Copy+accum
                # on Scalar -- each stage one tile behind the previous so no
                # engine ever stalls on the current tile's producers.
                drain(1, 1, 1)

            drain(0, 0, 0)

            # epilogue: pt_row -= ysel;  ce = sum ln(ysel + 1e-8)
            ly_strip = stpool.tile([128, N_TILES], F32)
            nc.scalar.activation(
                ly_strip[:], ys_strip[:], ACT.Ln, bias=eps1, scale=1.0
            )
            nc.vector.tensor_tensor(
                pt_strip[:], pt_strip[:], ys_strip[:], OP.subtract
            )

            ptsum = spool.tile([128, 1], F32, tag="ptsum")
            nc.vector.reduce_sum(ptsum[:], pt_strip[:], axis=AX.X)
            cesum = spool.tile([128, 1], F32, tag="cesum")
            nc.vector.reduce_sum(cesum[:], ly_strip[:], axis=AX.X)
            packed = spool.tile([128, 2], F32, tag="packed")
            nc.vector.tensor_copy(packed[:, 0:1], ptsum[:])
            nc.vector.tensor_copy(packed[:, 1:2], cesum[:])

            acc = pso.tile([1, 2], F32)
            nc.tensor.matmul(acc[:], ones1, packed[:], start=True, stop=True)
            sb_out = spool.tile([1, 2], F32, tag="sbout")
            nc.vector.tensor_copy(sb_out[:], acc[:])
            nc.sync.dma_start(out_d[:], sb_out[:])

    nc.compile()
    return nc


def _consts():
    import ml_dtypes

    pw = np.zeros((128, DIM), dtype=np.uint16)
    pw[:] = (1 << np.arange(DIM, dtype=np.uint16))[None, :]
    cbits = np.zeros((DIM + 2, N), dtype=np.float32)
    c = np.arange(N, dtype=np.uint32)
    pc = np.zeros(N, dtype=np.float32)
    for k in range(DIM):
        cbits[k, :] = (c >> k) & 1
        pc += cbits[k, :]
    cbits[DIM, :] = 1.0
    cbits[DIM + 1, :] = pc
    cbits = cbits.astype(ml_dtypes.bfloat16)
    ident = np.eye(128, dtype=np.float32)
    misc = np.zeros((128, 48), dtype=np.float32)
    for p in range(128):
        misc[p, p % 16] = 1.0
    misc[:, 16:26] = 0.0
    misc[:, 26] = 0.0
    misc[:, 27] = 1e-8
    misc[:, 28] = 1.0
    return pw, cbits, ident, misc


def kernel(y_true: np.ndarray, target: np.ndarray) -> np.ndarray:
    assert y_true.shape == (B_FULL, N) and target.shape == (B_FULL, N)
    if "nc" not in _cache:
        _cache["nc"] = _build_program()
    nc = _cache["nc"]

    pw, cbits, ident, misc = _consts()
    in_maps = []
    for c in range(N_CORES):
        sl = slice(c * B_SHARD, (c + 1) * B_SHARD)
        in_maps.append({
            "y_true": np.ascontiguousarray(y_true[sl]),
            "target": np.ascontiguousarray(target[sl]),
            "c_pow2": pw,
            "c_bits": cbits,
            "c_ident": ident,
            "c_misc": misc,
        })

    res = run_bass_kernel_spmd(nc, in_maps, core_ids=list(range(N_CORES)))
    _cache["last_results"] = res

    pt_sum = 0.0
    ce_sum = 0.0
    for c in range(N_CORES):
        o = res.results[c]["out"]
        pt_sum += float(o[0, 0])
        ce_sum += float(o[0, 1])
    loss = -ce_sum / B_FULL + pt_sum / (B_FULL * N)
    return np.float32(loss)
